# revision 2
# baseline (speedup 1.0000x reference)
"""EventRNN Trainium2 Bass kernel — v2: split-static-gates + DoubleRow fp8.

Full-input contract: kernel(**inputs) takes the complete arrays from
setup_inputs() and returns (h_new[None], c_new[None]).

Architecture vs the 59.3us baseline:
  * The LSTM weight [2048k, 2048c] was 8.4 MB bf16 replicated per core —
    half the per-core HBM traffic. Split: the 1536 k-rows multiplying
    host-known x parts (caption | feature | h_last) are column-sharded
    8 ways. Each core computes static gates [32b, 256c] for ALL batches
    from its slice, and an AllGather (input ready ~t7us, done ~t28us,
    hidden under the attention DMA stream) distributes the full
    [32, 2048] static gates. Only W_fc [512, 2048] (the fc-dependent
    k-rows) stays replicated: per-core W traffic 8.4 -> 2.9 MB.
  * logits and context matvecs run as fp8 DoubleRow matmuls (0.5
    cyc/row, 2 k-tiles per instruction): PE time for those 27 -> 6.8us.
    DR stationaries need 16B-aligned k-pair strides: w ships in a
    [128,2,2,2,16] padded layout and alphaT lives in [128,8,2,16].
  * both halves pipeline through ONE batch loop (preloads first, DR
    matmuls in arrival order, softmax/alphaT/ctx one batch behind) so
    the PE in-order stream never stalls on a later DMA; fc rows are
    assembled per batch (scale = beta/sum rides the ACT psum->sbuf
    copy as a per-partition AP) and transposed into fcT immediately.
  * relu(proj+q) rotates DVE/Pool/ACT per chunk (ACT via
    activation(Relu, bias=q_col)); batch 0 leans on ACT (idle then),
    later batches on DVE/Pool while ACT runs the exp stream.
  * static gates re-enter PSUM through a sel[32,4] one-hot matmul; the
    last W_fc k-chunk ships as four column DMAs ordered [f,i,o,g] so
    the f-gate matmul and the LSTM tail start as the bytes land.
"""

import numpy as np

import concourse.bacc as bacc
import concourse.mybir as mybir
import concourse.tile as tile
from concourse.bass_utils import run_bass_kernel_spmd

F32 = mybir.dt.float32
BF16 = mybir.dt.bfloat16
FP8 = mybir.dt.float8e4
AF = mybir.ActivationFunctionType
ALU = mybir.AluOpType
DR = mybir.MatmulPerfMode.DoubleRow

B, L, D, H = 32, 2048, 512, 512
N_CORES = 8
B_LOC = B // N_CORES          # 4 batches per core
FIDX = 1024                   # static feature_idx from setup_inputs()
HALF = L // 2
P = 128
DC = D // P                   # 4 d-chunks
LC = HALF // P                # 8 l-chunks per half
G4 = 4 * H                    # 2048 gate columns
CSL = G4 // N_CORES           # 256 static-gate columns per core
KST = (H + 2 * D) // P        # 12 static k-chunks (cap|feat|h)
WSCALE = 64.0                 # fp8 scale for w_patt/w_fatt (and madd)
PROWS = 32 * (B_LOC - 1) + 1  # 97: per-batch rows live at partition 32*b


def build_nc():
    nc = bacc.Bacc("TRN2", target_bir_lowering=False, debug=False,
                   num_devices=N_CORES)

    # ---- DRAM I/O (per-core shards; host prep is layout + quantization) ----
    projT = nc.dram_tensor("projT", [B_LOC, D, L], FP8, kind="ExternalInput").ap()
    feats = nc.dram_tensor("feats", [B_LOC, L, D], FP8, kind="ExternalInput").ap()
    WfcT = nc.dram_tensor("WfcT", [D, G4], BF16, kind="ExternalInput").ap()
    Wst = nc.dram_tensor("Wst", [KST * P, CSL], BF16, kind="ExternalInput").ap()
    Xst = nc.dram_tensor("Xst", [KST * P, B], BF16, kind="ExternalInput").ap()
    wbias = nc.dram_tensor("wbias", [1, CSL], BF16, kind="ExternalInput").ap()
    sel = nc.dram_tensor("sel", [B, B_LOC], BF16, kind="ExternalInput").ap()
    w_h2aT = nc.dram_tensor("w_h2aT", [H, D], FP8, kind="ExternalInput").ap()
    wdr = nc.dram_tensor("wdr", [P, 2 * 2 * 2 * 16], FP8, kind="ExternalInput").ap()
    pack = nc.dram_tensor("pack", [D, 7], BF16, kind="ExternalInput").ap()
    maskT = nc.dram_tensor("maskT", [P, 2 * (LC // 2) * 2 * B_LOC], BF16,
                           kind="ExternalInput").ap()
    c_last = nc.dram_tensor("c_last", [B_LOC, H], F32, kind="ExternalInput").ap()
    h_out = nc.dram_tensor("h_new", [B_LOC, H], F32, kind="ExternalOutput").ap()
    c_out = nc.dram_tensor("c_new", [B_LOC, H], F32, kind="ExternalOutput").ap()

    with tile.TileContext(nc) as tc:
        with tc.tile_pool(name="const", bufs=1) as const, \
             tc.tile_pool(name="wfc", bufs=3) as wfc, \
             tc.tile_pool(name="wfc3", bufs=4) as wfc3p, \
             tc.tile_pool(name="dram", bufs=1, space="DRAM") as dram:

            # ---- resident tiles ----
            ones_bf = const.tile([1, B], BF16)
            nc.gpsimd.memset(ones_bf[:], 1.0)
            identf = const.tile([1, 1], F32)
            nc.gpsimd.memset(identf[:], 1.0)
            ones128 = const.tile([1, P], BF16)
            nc.gpsimd.memset(ones128[:], 1.0)
            zeros512 = const.tile([1, D], BF16)
            nc.gpsimd.memset(zeros512[:], 0.0)
            # fp8 DR ones stationary for the denominator column sums
            onesdr = const.tile([P, 2, 16], FP8)
            nc.gpsimd.memset(onesdr[:], 1.0)

            pack_sb = const.tile([P, DC, 7], BF16)
            w_h2aT_sb = const.tile([P, H // P, D], FP8)
            wdr_sb = const.tile([P, 2, 2, 2, 16], FP8)
            maskT_sb = const.tile([P, 2, LC // 2, 2, B_LOC], BF16)
            c_last_sb = const.tile([B_LOC, H], F32)
            sel_sb = const.tile([B, B_LOC], BF16)
            Xst_sb = const.tile([P, KST, B], BF16)
            Wst_sb = const.tile([P, KST, CSL], BF16)
            wb_sb = const.tile([1, CSL], BF16)
            gs_loc = const.tile([B, CSL], BF16)
            gs_sb = const.tile([B, G4], BF16)
            # last W_fc k-chunk ships as 4 column tiles (f first)
            WfcT_sb = [wfc.tile([P, G4], BF16, name=f"wfc{k}")
                       for k in range(DC - 1)]
            Wfc3_sb = [wfc3p.tile([P, 512], BF16, name=f"wfc3c{c}")
                       for c in range(4)]

            qb = const.tile([P, DC * B_LOC], F32)
            b_h2a_f = const.tile([P, DC], F32)
            onePlus = const.tile([1, B_LOC], F32)
            sv = const.tile([1, 2, B_LOC], F32)
            alphaT = const.tile([P, 2 * LC // 2, 2, 16], FP8)  # [p, hl, m, slot]
            fcrow_t = const.tile([1, H], F32)
            sigwarm = const.tile([1, 1], F32)
            fcrow = const.tile([1, H], F32)
            fcT_sb = const.tile([P, DC, B_LOC], BF16)

            gin_b = dram.tile([B, CSL], BF16)
            gout_b = dram.tile([N_CORES * B, CSL], BF16)

            from contextlib import ExitStack
            stream_ctx = ExitStack()
            projp = stream_ctx.enter_context(tc.tile_pool(name="proj", bufs=B_LOC))
            fpool = stream_ctx.enter_context(tc.tile_pool(name="fpool", bufs=B_LOC))
            hattp = stream_ctx.enter_context(tc.tile_pool(name="hatt", bufs=4))
            projt, fq = {}, {}
            for b in range(B_LOC):
                projt[b] = projp.tile([P, DC, L], FP8, name="projt")
                fq[b] = fpool.tile([P, 2 * LC, D], FP8, name="fq")

            # ---- DMA queue (SP, served in emission order): q deps,
            # proj0, static-gates deps, smalls; phase A and the static
            # matmuls are emitted inline so the gin bounce (which reads
            # gs_loc) sits after proj0 without blocking the queue head ----
            nc.sync.dma_start(pack_sb[:], pack.rearrange("(c p) n -> p c n", p=P))
            nc.sync.dma_start(w_h2aT_sb[:],
                              w_h2aT.rearrange("(c p) n -> p c n", p=P))
            nc.sync.dma_start(Xst_sb[:], Xst.rearrange("(c p) n -> p c n", p=P))
            nc.sync.dma_start(Wst_sb[:], Wst.rearrange("(c p) n -> p c n", p=P))
            nc.sync.dma_start(wb_sb[:], wbias[:])
            nc.sync.dma_start(projt[0][:],
                              projT[0].rearrange("(c p) l -> p c l", p=P))
            nc.sync.dma_start(wdr_sb[:].rearrange("p a b c d -> p (a b c d)"),
                              wdr[:])
            nc.sync.dma_start(
                maskT_sb[:].rearrange("p a b c d -> p (a b c d)"), maskT[:])
            nc.sync.dma_start(c_last_sb[:], c_last[:])
            nc.sync.dma_start(sel_sb[:], sel[:])

            # ============ phase A: q and beta (emitted first: q warms PE
            # while the static weights stream in) ============
            with tc.tile_pool(name="psA", bufs=1, space="PSUM") as psA:
                q_ps = psA.tile([P, DC * B_LOC], F32)
                beta_ps = psA.tile([1, B_LOC], F32)
                nc.vector.tensor_copy(b_h2a_f[:], pack_sb[:, :, 5])
                for dc in range(DC):
                    for kc in range(H // P):
                        nc.tensor.matmul(
                            q_ps[:, dc * B_LOC:(dc + 1) * B_LOC],
                            w_h2aT_sb[:, kc, dc * P:(dc + 1) * P],
                            pack_sb[:, kc, 0:4],
                            start=(kc == 0), stop=(kc == H // P - 1))
                    nc.vector.tensor_scalar(
                        qb[:, dc * B_LOC:(dc + 1) * B_LOC],
                        q_ps[:, dc * B_LOC:(dc + 1) * B_LOC],
                        1.0 / WSCALE, b_h2a_f[:, dc:dc + 1],
                        op0=ALU.mult, op1=ALU.add)
                for b in range(B_LOC):
                    for kc in range(H // P):
                        nc.tensor.matmul(beta_ps[0:1, b:b + 1],
                                         pack_sb[:, kc, 4:5],
                                         pack_sb[:, kc, b:b + 1],
                                         start=(kc == 0),
                                         stop=(kc == H // P - 1))
                # 1/beta - 1 = exp(-x - b_sel); fc scale folds beta via
                # s = 1 / (sum * (1 + e))
                nc.scalar.activation(onePlus[:], beta_ps[0:1, :],
                                     AF.Exp, scale=-1.0,
                                     bias=pack_sb[0:1, 0, 6:7])
                nc.vector.tensor_scalar(onePlus[:], onePlus[:], 1.0, None,
                                        op0=ALU.add)

            # ============ phase S: static gates + AllGather ============
            with tc.tile_pool(name="psS", bufs=1, space="PSUM") as psS:
                gs_ps = psS.tile([B, CSL], F32)
                # bias row opens the psum group (bias bcast to all 32 rows)
                nc.tensor.matmul(gs_ps[:], ones_bf[0:1, :], wb_sb[0:1, :],
                                 start=True, stop=False)
                for kc in range(KST):
                    nc.tensor.matmul(gs_ps[:], Xst_sb[:, kc, :], Wst_sb[:, kc, :],
                                     start=False, stop=(kc == KST - 1))
                nc.vector.tensor_copy(gs_loc[:], gs_ps[:])
            # by the time the in-order SP queue reaches this, gs_loc is ready
            nc.sync.dma_start(gin_b[:], gs_loc[:])
            nc.gpsimd.collective_compute(
                "AllGather", ALU.bypass,
                replica_groups=[list(range(N_CORES))],
                ins=[gin_b.opt()], outs=[gout_b.opt()])

            nc.sync.dma_start(fq[0][:],
                              feats[0].rearrange("(j p) d -> p j d", p=P))
            for b in range(1, B_LOC):
                nc.sync.dma_start(projt[b][:],
                                  projT[b].rearrange("(c p) l -> p c l", p=P))
                nc.sync.dma_start(fq[b][:],
                                  feats[b].rearrange("(j p) d -> p j d", p=P))
            # collective readback before the W_fc stream: the static-gates
            # inject opens the psum regions, so gs must land before the
            # first fc matmul could run
            nc.sync.dma_start(
                gs_sb[:].rearrange("b (i n) -> b i n", i=N_CORES),
                gout_b[:].rearrange("(i b) n -> b i n", i=N_CORES))
            for k in range(DC - 1):
                nc.sync.dma_start(WfcT_sb[k][:], WfcT[k * P:(k + 1) * P, :])
            for cc in (1, 3, 0, 2):    # tail needs f, then g, i; o last
                nc.sync.dma_start(Wfc3_sb[cc][:],
                                  WfcT[3 * P:4 * P, cc * 512:(cc + 1) * 512])

            # ============ phase B: attention, both halves pipelined ========
            # logits are produced TRANSPOSED: DR matmuls with the hatt chunk
            # as the stationary pair give lgT[128l, (h,lcp,m,b)] at psum
            # partition base 0 (DoubleRow dst must be partition 0). exp then
            # runs on [128, 8] tiles and writes the fp8 alphaT directly; the
            # mask is a tiny additive DVE op on the transposed logits.
            with tc.tile_pool(name="pslog", bufs=1, space="PSUM") as pslog, \
                 tc.tile_pool(name="pssum", bufs=1, space="PSUM") as pssum, \
                 tc.tile_pool(name="psctx", bufs=3, space="PSUM") as psctx, \
                 tc.tile_pool(name="psfc", bufs=1, space="PSUM") as psfc:
                lgT = pslog.tile([P, 2, LC // 2, 2, B_LOC], F32)
                sums_ps = pssum.tile([1, 2, B_LOC], F32)
                fcT_ps = psfc.tile([P, DC, B_LOC], F32)

                # zero-openers (complete groups; everything accumulates on
                # top with skip_group_check)
                nc.tensor.matmul(
                    lgT[:].rearrange("p a b c d -> p (a b c d)"),
                    ones128[0:1, :], zeros512[0:1, 0:2 * LC * B_LOC],
                    start=True, stop=True)
                nc.tensor.matmul(
                    sums_ps[:].rearrange("p a b -> p (a b)"),
                    ones_bf[0:1, 0:1], zeros512[0:1, 0:2 * B_LOC],
                    start=True, stop=True)
                nc.tensor.matmul(
                    fcT_ps[:].rearrange("p a b -> p (a b)"), ones128[0:1, :],
                    zeros512[0:1, 0:DC * B_LOC], start=True, stop=True)

                def relu_logits(h, b):
                    hatt8 = hattp.tile([P, DC, HALF], FP8)
                    for dc in range(DC):
                        src = projt[b][:, dc, h * HALF:(h + 1) * HALF]
                        qcol = qb[:, dc * B_LOC + b:dc * B_LOC + b + 1]
                        # per-batch balance: b0 avoids Pool (its queue is
                        # head-blocked by the collective's input wait); b3
                        # mostly avoids Pool so the last chain stays short
                        if b == 0:
                            eng = (("v", "a", "a", "v"), ("a", "v", "v", "a"))[h][dc]
                        elif b == B_LOC - 1:
                            eng = (("v", "a", "p", "v"), ("a", "v", "a", "p"))[h][dc]
                        elif h == 0:
                            eng = ("v", "p", "a", "v")[dc]
                        else:
                            eng = ("p", "v", "a", "p")[dc]
                        if eng == "p":
                            nc.gpsimd.tensor_scalar(
                                hatt8[:, dc, :], src, qcol, 0.0,
                                op0=ALU.add, op1=ALU.max)
                        elif eng == "a":
                            nc.scalar.activation(hatt8[:, dc, :], src,
                                                 AF.Relu, bias=qcol)
                        else:
                            nc.vector.tensor_scalar(
                                hatt8[:, dc, :], src, qcol, 0.0,
                                op0=ALU.add, op1=ALU.max)
                    # transposed logits: stationary = hatt d-pair x 128 l
                    # cols, moving = the padded w pair column
                    for lcp in range(LC // 2):
                        for m in range(2):
                            lc = 2 * lcp + m
                            for dcp in range(2):
                                nc.tensor.matmul(
                                    lgT[:, h, lcp, m, b:b + 1],
                                    hatt8[:, 2 * dcp:2 * dcp + 2,
                                          lc * P:(lc + 1) * P],
                                    wdr_sb[:, h, dcp, :, 0:1],
                                    start=False, stop=(dcp == 1),
                                    skip_group_check=True,
                                    perf_mode=DR)

                def softmax_ctx(b):
                    # mask, exp->fp8 alphaT, beta/sum fold, unnormalized ctx,
                    # scaled fc rows, accumulating transposes into fcT; all
                    # per-batch rows live at partition 0 now
                    for h in range(2):
                        nc.vector.tensor_tensor(
                            lgT[:, h, :, :, b], lgT[:, h, :, :, b],
                            maskT_sb[:, h, :, :, b], op=ALU.add)
                        nc.scalar.activation(
                            alphaT[:, 4 * h:4 * h + 4, :, b:b + 1],
                            lgT[:, h, :, :, b:b + 1],
                            AF.Exp, scale=1.0 / WSCALE)
                        for j in range(LC // 2):
                            nc.tensor.matmul(
                                sums_ps[0:1, h, b:b + 1],
                                onesdr[:, :, 0:1],
                                alphaT[:, 4 * h + j, :, b:b + 1],
                                start=False, stop=(j == LC // 2 - 1),
                                skip_group_check=True, perf_mode=DR)
                    nc.vector.tensor_scalar(sv[0:1, :, b], sums_ps[0:1, :, b],
                                            onePlus[0:1, b:b + 1], None,
                                            op0=ALU.mult)
                    nc.vector.reciprocal(sv[0:1, :, b], sv[0:1, :, b])
                    for h, row in ((0, fcrow), (1, fcrow_t)):
                        ctx_t = psctx.tile([1, D], F32, name="ctxt")
                        for j in range(LC // 2):
                            nc.tensor.matmul(
                                ctx_t[:],
                                alphaT[:, 4 * h + j, :, b:b + 1],
                                fq[b][:, 8 * h + 2 * j:8 * h + 2 * j + 2, :],
                                start=(j == 0), stop=(j == LC // 2 - 1),
                                perf_mode=DR)
                        if h == 0:
                            # ACT has slack now; DVE was pacing the chain
                            nc.scalar.activation(row[:], ctx_t[:], AF.Copy,
                                                 scale=sv[0:1, h, b:b + 1])
                        else:
                            nc.vector.tensor_scalar(row[:], ctx_t[:],
                                                    sv[0:1, h, b:b + 1], None,
                                                    op0=ALU.mult)
                        for dc in range(DC):
                            nc.tensor.matmul(
                                fcT_ps[:, dc, b:b + 1],
                                row[0:1, dc * P:(dc + 1) * P],
                                identf[:], is_transpose=True,
                                start=False, stop=(h == 1),
                                skip_group_check=True)

                for b in range(B_LOC):
                    if b >= 1:
                        softmax_ctx(b - 1)
                    relu_logits(0, b)
                    relu_logits(1, b)
                softmax_ctx(B_LOC - 1)
                nc.vector.tensor_copy(fcT_sb[:], fcT_ps[:])
                # all Exp-table ACT work is done; a dummy Sigmoid op
                # (reading the last exp's output so the scheduler pins it
                # right here) hoists the Sigmoid table load off the tail
                nc.scalar.activation(sigwarm[:], alphaT[0:1, 7, 1, 3:4],
                                     AF.Sigmoid)

            # streaming pools close here; the tail pool reuses their SBUF
            stream_ctx.close()

            # ============ phase C: gates + LSTM tail ============
            with tc.tile_pool(name="psg", bufs=1, space="PSUM") as psgp, \
                 tc.tile_pool(name="tailp", bufs=1) as tailp:
                # one psum tile per gate segment: each sigmoid then waits
                # only on its own segment's matmuls (deps are tile-granular)
                psgt = [psgp.tile([B_LOC, 512], F32, name=f"psg{c}")
                        for c in range(4)]
                for cc in (1, 3, 0, 2):   # close f, then g, i, o (tail order)
                    nc.tensor.matmul(psgt[cc][:], sel_sb[:],
                                     gs_sb[:, cc * 512:(cc + 1) * 512],
                                     start=True, stop=True)
                    for kc in range(DC):
                        mv = (WfcT_sb[kc][:, cc * 512:(cc + 1) * 512]
                              if kc < DC - 1 else Wfc3_sb[cc][:])
                        nc.tensor.matmul(
                            psgt[cc][:], fcT_sb[:, kc, :], mv,
                            start=False, stop=(kc == DC - 1),
                            skip_group_check=True)

                # gate columns are host-permuted to [i, f, o, g]
                g_sb = tailp.tile([B_LOC, G4], BF16)
                nc.scalar.activation(g_sb[:, H:2 * H], psgt[1][:],
                                     AF.Sigmoid)
                c1 = tailp.tile([B_LOC, H], F32)
                nc.vector.tensor_tensor(c1[:], g_sb[:, H:2 * H],
                                        c_last_sb[:], op=ALU.mult)
                # tanh(x) = 2*sigmoid(2x) - 1 (stays on the Sigmoid table)
                nc.scalar.activation(g_sb[:, 3 * H:4 * H], psgt[3][:],
                                     AF.Sigmoid, scale=2.0)
                nc.vector.tensor_scalar(g_sb[:, 3 * H:4 * H],
                                        g_sb[:, 3 * H:4 * H], 2.0, -1.0,
                                        op0=ALU.mult, op1=ALU.add)
                nc.scalar.activation(g_sb[:, 0:H], psgt[0][:], AF.Sigmoid)
                nc.scalar.activation(g_sb[:, 2 * H:3 * H], psgt[2][:],
                                     AF.Sigmoid)

                t2 = tailp.tile([B_LOC, H], BF16)
                tf = tailp.tile([B_LOC, H], F32)
                c_new = tailp.tile([B_LOC, H], F32)
                h_new = tailp.tile([B_LOC, H], F32)
                nc.vector.tensor_tensor(t2[:], g_sb[:, 0:H],
                                        g_sb[:, 3 * H:4 * H], op=ALU.mult)
                nc.vector.tensor_tensor(c_new[:], c1[:], t2[:], op=ALU.add)
                nc.sync.dma_start(c_out[:], c_new[:])
                nc.scalar.activation(tf[:], c_new[:], AF.Sigmoid, scale=2.0)
                nc.vector.tensor_scalar(tf[:], tf[:], 2.0, -1.0,
                                        op0=ALU.mult, op1=ALU.add)
                nc.vector.tensor_tensor(h_new[:], g_sb[:, 2 * H:3 * H], tf[:],
                                        op=ALU.mult)
                nc.scalar.dma_start(h_out[:], h_new[:])

    nc.compile()
    return nc


_NC_CACHE = None


def _get_nc():
    global _NC_CACHE
    if _NC_CACHE is None:
        _NC_CACHE = build_nc()
    return _NC_CACHE


def make_in_maps(features, features_proj, hidden_states, cell_states,
                 caption_hidden_states, w_h2a, b_h2a, w_patt, b_patt,
                 w_fatt, b_fatt, w_sel, b_sel, w_ih, w_hh, b_ih, b_hh,
                 mask, feature_idx):
    assert int(feature_idx) == FIDX
    import ml_dtypes
    f32 = np.float32
    bf16 = ml_dtypes.bfloat16
    fp8 = ml_dtypes.float8_e4m3
    features = np.asarray(features, f32)
    features_proj = np.asarray(features_proj, f32)
    h_last = np.asarray(hidden_states, f32)[-1]          # [B, H]
    c_lastv = np.asarray(cell_states, f32)[-1]           # [B, H]
    cap = np.asarray(caption_hidden_states, f32)         # [B, H]
    mask = np.asarray(mask)

    # fused LSTM weight, gate columns permuted [i, f, o, g]
    Wfull = np.concatenate([np.asarray(w_ih, f32), np.asarray(w_hh, f32)],
                           axis=1)                       # [2048c, 2048k]
    gate_perm = np.r_[0:512, 512:1024, 1536:2048, 1024:1536]
    WTp = np.ascontiguousarray(Wfull[gate_perm].T)       # [2048k, 2048c]
    biasv = (np.asarray(b_ih, f32) + np.asarray(b_hh, f32))[gate_perm]
    st_rows = np.r_[0:512, 1024:2048]                    # cap | feat | h rows
    WfcT = np.ascontiguousarray(WTp[512:1024]).astype(bf16)
    Wstat = WTp[st_rows]                                 # [1536, 2048]
    # static x for ALL batches: [capT; featT; hT]  [1536, 32]
    XstV = np.concatenate([cap.T, features[:, FIDX, :].T, h_last.T],
                          axis=0).astype(bf16)

    w_h2aTv = np.ascontiguousarray(
        np.asarray(w_h2a, f32).T * WSCALE).astype(fp8)
    # DR-padded attention weights: [p, h, dcp, m, 16], value at slot 0
    wdrv = np.zeros((P, 2, 2, 2, 16), f32)
    watt = np.stack([np.asarray(w_patt, f32)[0], np.asarray(w_fatt, f32)[0]])
    for h in range(2):
        for dcp in range(2):
            for m in range(2):
                dc = 2 * dcp + m
                wdrv[:, h, dcp, m, 0] = watt[h, dc * P:(dc + 1) * P] * WSCALE
    wdrv = wdrv.reshape(P, -1).astype(fp8)

    # additive mask in transposed-logit layout [p, h, lcp, m, b]; the
    # per-half attention bias b_att cancels in the softmax and is dropped
    madd = np.where(mask.reshape(B, 2, HALF), f32(0.0), f32(-1e30)) * WSCALE
    # [B, h, lcp, m, p] -> [p, h, lcp, m, B]
    maskTv = madd.reshape(B, 2, LC // 2, 2, P).transpose(4, 1, 2, 3, 0)

    in_maps = []
    for c in range(N_CORES):
        sl = slice(c * B_LOC, (c + 1) * B_LOC)
        packv = np.zeros((D, 7), f32)
        packv[:, 0:4] = h_last[sl].T
        packv[:, 4] = np.asarray(w_sel, f32)[0]
        packv[:, 5] = np.asarray(b_h2a, f32)
        packv[:, 6] = -np.asarray(b_sel, f32)[0]
        selv = np.zeros((B, B_LOC), f32)
        for j in range(B_LOC):
            selv[c * B_LOC + j, j] = 1.0
        in_maps.append({
            "projT": np.ascontiguousarray(
                features_proj[sl].transpose(0, 2, 1)).astype(fp8),
            "feats": np.ascontiguousarray(features[sl]).astype(fp8),
            "WfcT": WfcT,
            "Wst": np.ascontiguousarray(
                Wstat[:, c * CSL:(c + 1) * CSL]).astype(bf16),
            "Xst": XstV,
            "wbias": np.ascontiguousarray(
                biasv[None, c * CSL:(c + 1) * CSL]).astype(bf16),
            "sel": selv.astype(bf16),
            "w_h2aT": w_h2aTv,
            "wdr": wdrv,
            "pack": np.ascontiguousarray(packv).astype(bf16),
            "maskT": np.ascontiguousarray(
                maskTv[:, :, :, :, sl].reshape(P, -1)).astype(bf16),
            "c_last": np.ascontiguousarray(c_lastv[sl]),
        })
    return in_maps


def run(trace=False, **inputs):
    nc = _get_nc()
    in_maps = make_in_maps(**inputs)
    res = run_bass_kernel_spmd(nc, in_maps, core_ids=list(range(N_CORES)),
                               trace=trace)
    h = np.concatenate([res.results[c]["h_new"] for c in range(N_CORES)], axis=0)
    c = np.concatenate([res.results[c]["c_new"] for c in range(N_CORES)], axis=0)
    return (h[None], c[None]), res


def kernel(**inputs):
    out, _ = run(trace=False, **inputs)
    return out


# revision 3
# speedup vs baseline: 1.0173x; 1.0173x over previous
"""EventRNN Trainium2 Bass kernel — v2: split-static-gates + DoubleRow fp8.

Full-input contract: kernel(**inputs) takes the complete arrays from
setup_inputs() and returns (h_new[None], c_new[None]).

Architecture vs the 59.3us baseline:
  * The LSTM weight [2048k, 2048c] was 8.4 MB bf16 replicated per core —
    half the per-core HBM traffic. Split: the 1536 k-rows multiplying
    host-known x parts (caption | feature | h_last) are column-sharded
    8 ways. Each core computes static gates [32b, 256c] for ALL batches
    from its slice, and an AllGather (input ready ~t7us, done ~t28us,
    hidden under the attention DMA stream) distributes the full
    [32, 2048] static gates. Only W_fc [512, 2048] (the fc-dependent
    k-rows) stays replicated: per-core W traffic 8.4 -> 2.9 MB.
  * logits and context matvecs run as fp8 DoubleRow matmuls (0.5
    cyc/row, 2 k-tiles per instruction): PE time for those 27 -> 6.8us.
    DR stationaries need 16B-aligned k-pair strides: w ships in a
    [128,2,2,2,16] padded layout and alphaT lives in [128,8,2,16].
  * both halves pipeline through ONE batch loop (preloads first, DR
    matmuls in arrival order, softmax/alphaT/ctx one batch behind) so
    the PE in-order stream never stalls on a later DMA; fc rows are
    assembled per batch (scale = beta/sum rides the ACT psum->sbuf
    copy as a per-partition AP) and transposed into fcT immediately.
  * relu(proj+q) rotates DVE/Pool/ACT per chunk (ACT via
    activation(Relu, bias=q_col)); batch 0 leans on ACT (idle then),
    later batches on DVE/Pool while ACT runs the exp stream.
  * static gates re-enter PSUM through a sel[32,4] one-hot matmul; the
    last W_fc k-chunk ships as four column DMAs ordered [f,i,o,g] so
    the f-gate matmul and the LSTM tail start as the bytes land.
"""

import numpy as np

import concourse.bacc as bacc
import concourse.mybir as mybir
import concourse.tile as tile
from concourse.bass_utils import run_bass_kernel_spmd

F32 = mybir.dt.float32
BF16 = mybir.dt.bfloat16
FP8 = mybir.dt.float8e4
AF = mybir.ActivationFunctionType
ALU = mybir.AluOpType
DR = mybir.MatmulPerfMode.DoubleRow

B, L, D, H = 32, 2048, 512, 512
N_CORES = 8
B_LOC = B // N_CORES          # 4 batches per core
FIDX = 1024                   # static feature_idx from setup_inputs()
HALF = L // 2
P = 128
DC = D // P                   # 4 d-chunks
LC = HALF // P                # 8 l-chunks per half
G4 = 4 * H                    # 2048 gate columns
CSL = G4 // N_CORES           # 256 static-gate columns per core
KST = (H + 2 * D) // P        # 12 static k-chunks (cap|feat|h)
WSCALE = 64.0                 # fp8 scale for w_patt/w_fatt (and madd)
PROWS = 32 * (B_LOC - 1) + 1  # 97: per-batch rows live at partition 32*b


def build_nc():
    nc = bacc.Bacc("TRN2", target_bir_lowering=False, debug=False,
                   num_devices=N_CORES)

    # ---- DRAM I/O (per-core shards; host prep is layout + quantization) ----
    projT = nc.dram_tensor("projT", [B_LOC, D, L], FP8, kind="ExternalInput").ap()
    feats = nc.dram_tensor("feats", [B_LOC, L, D], FP8, kind="ExternalInput").ap()
    WfcT = nc.dram_tensor("WfcT", [D, G4], BF16, kind="ExternalInput").ap()
    Wst = nc.dram_tensor("Wst", [KST * P, CSL], BF16, kind="ExternalInput").ap()
    Xst = nc.dram_tensor("Xst", [KST * P, B], BF16, kind="ExternalInput").ap()
    wbias = nc.dram_tensor("wbias", [1, CSL], BF16, kind="ExternalInput").ap()
    sel = nc.dram_tensor("sel", [B, B_LOC], BF16, kind="ExternalInput").ap()
    w_h2aT = nc.dram_tensor("w_h2aT", [H, D], FP8, kind="ExternalInput").ap()
    wdr = nc.dram_tensor("wdr", [P, 2 * 2 * 2 * 16], FP8, kind="ExternalInput").ap()
    pack = nc.dram_tensor("pack", [D, 7], BF16, kind="ExternalInput").ap()
    maskT = nc.dram_tensor("maskT", [P, 2 * (LC // 2) * 2 * B_LOC], BF16,
                           kind="ExternalInput").ap()
    c_last = nc.dram_tensor("c_last", [B_LOC, H], F32, kind="ExternalInput").ap()
    h_out = nc.dram_tensor("h_new", [B_LOC, H], F32, kind="ExternalOutput").ap()
    c_out = nc.dram_tensor("c_new", [B_LOC, H], F32, kind="ExternalOutput").ap()

    with tile.TileContext(nc) as tc:
        with tc.tile_pool(name="const", bufs=1) as const, \
             tc.tile_pool(name="wfc", bufs=3) as wfc, \
             tc.tile_pool(name="wfc3", bufs=4) as wfc3p, \
             tc.tile_pool(name="dram", bufs=1, space="DRAM") as dram:

            # ---- resident tiles ----
            ones_bf = const.tile([1, B], BF16)
            nc.gpsimd.memset(ones_bf[:], 1.0)
            identf = const.tile([1, 1], F32)
            nc.gpsimd.memset(identf[:], 1.0)
            ones128 = const.tile([1, P], BF16)
            nc.gpsimd.memset(ones128[:], 1.0)
            zeros512 = const.tile([1, D], BF16)
            nc.gpsimd.memset(zeros512[:], 0.0)
            # fp8 DR ones stationary for the denominator column sums
            onesdr = const.tile([P, 2, 16], FP8)
            nc.gpsimd.memset(onesdr[:], 1.0)

            pack_sb = const.tile([P, DC, 7], BF16)
            w_h2aT_sb = const.tile([P, H // P, D], FP8)
            wdr_sb = const.tile([P, 2, 2, 2, 16], FP8)
            maskT_sb = const.tile([P, 2, LC // 2, 2, B_LOC], BF16)
            c_last_sb = const.tile([B_LOC, H], F32)
            sel_sb = const.tile([B, B_LOC], BF16)
            Xst_sb = const.tile([P, KST, B], BF16)
            Wst_sb = const.tile([P, KST, CSL], BF16)
            wb_sb = const.tile([1, CSL], BF16)
            gs_loc = const.tile([B, CSL], BF16)
            gs_sb = const.tile([B, G4], BF16)
            # last W_fc k-chunk ships as 4 column tiles (f first)
            WfcT_sb = [wfc.tile([P, G4], BF16, name=f"wfc{k}")
                       for k in range(DC - 1)]
            Wfc3_sb = [wfc3p.tile([P, 512], BF16, name=f"wfc3c{c}")
                       for c in range(4)]

            qb = const.tile([P, DC * B_LOC], F32)
            b_h2a_f = const.tile([P, DC], F32)
            onePlus = const.tile([1, B_LOC], F32)
            sv = const.tile([1, 2, B_LOC], F32)
            alphaT = const.tile([P, 2 * LC // 2, 2, 16], FP8)  # [p, hl, m, slot]
            fcrow_t = const.tile([1, H], F32)
            sigwarm = const.tile([1, 1], F32)
            fcrow = const.tile([1, H], F32)
            fcT_sb = const.tile([P, DC, B_LOC], BF16)

            gin_b = dram.tile([B, CSL], BF16)
            gout_b = dram.tile([N_CORES * B, CSL], BF16)

            from contextlib import ExitStack
            stream_ctx = ExitStack()
            projp = stream_ctx.enter_context(tc.tile_pool(name="proj", bufs=B_LOC))
            fpool = stream_ctx.enter_context(tc.tile_pool(name="fpool", bufs=B_LOC))
            hattp = stream_ctx.enter_context(tc.tile_pool(name="hatt", bufs=4))
            projt, fq = {}, {}
            for b in range(B_LOC):
                projt[b] = projp.tile([P, DC, L], FP8, name="projt")
                fq[b] = fpool.tile([P, 2 * LC, D], FP8, name="fq")

            # ---- DMA queue (SP, served in emission order): q deps,
            # proj0, static-gates deps, smalls; phase A and the static
            # matmuls are emitted inline so the gin bounce (which reads
            # gs_loc) sits after proj0 without blocking the queue head ----
            nc.sync.dma_start(pack_sb[:], pack.rearrange("(c p) n -> p c n", p=P))
            nc.sync.dma_start(w_h2aT_sb[:],
                              w_h2aT.rearrange("(c p) n -> p c n", p=P))
            nc.sync.dma_start(Xst_sb[:], Xst.rearrange("(c p) n -> p c n", p=P))
            nc.sync.dma_start(Wst_sb[:], Wst.rearrange("(c p) n -> p c n", p=P))
            nc.sync.dma_start(wb_sb[:], wbias[:])
            nc.sync.dma_start(projt[0][:],
                              projT[0].rearrange("(c p) l -> p c l", p=P))
            nc.sync.dma_start(wdr_sb[:].rearrange("p a b c d -> p (a b c d)"),
                              wdr[:])
            nc.sync.dma_start(
                maskT_sb[:].rearrange("p a b c d -> p (a b c d)"), maskT[:])
            nc.sync.dma_start(c_last_sb[:], c_last[:])
            nc.sync.dma_start(sel_sb[:], sel[:])

            # ============ phase A: q and beta (emitted first: q warms PE
            # while the static weights stream in) ============
            with tc.tile_pool(name="psA", bufs=1, space="PSUM") as psA:
                q_ps = psA.tile([P, DC * B_LOC], F32)
                beta_ps = psA.tile([1, B_LOC], F32)
                nc.vector.tensor_copy(b_h2a_f[:], pack_sb[:, :, 5])
                for dc in range(DC):
                    for kc in range(H // P):
                        nc.tensor.matmul(
                            q_ps[:, dc * B_LOC:(dc + 1) * B_LOC],
                            w_h2aT_sb[:, kc, dc * P:(dc + 1) * P],
                            pack_sb[:, kc, 0:4],
                            start=(kc == 0), stop=(kc == H // P - 1))
                    nc.vector.tensor_scalar(
                        qb[:, dc * B_LOC:(dc + 1) * B_LOC],
                        q_ps[:, dc * B_LOC:(dc + 1) * B_LOC],
                        1.0 / WSCALE, b_h2a_f[:, dc:dc + 1],
                        op0=ALU.mult, op1=ALU.add)
                for b in range(B_LOC):
                    for kc in range(H // P):
                        nc.tensor.matmul(beta_ps[0:1, b:b + 1],
                                         pack_sb[:, kc, 4:5],
                                         pack_sb[:, kc, b:b + 1],
                                         start=(kc == 0),
                                         stop=(kc == H // P - 1))
                # 1/beta - 1 = exp(-x - b_sel); fc scale folds beta via
                # s = 1 / (sum * (1 + e))
                nc.scalar.activation(onePlus[:], beta_ps[0:1, :],
                                     AF.Exp, scale=-1.0,
                                     bias=pack_sb[0:1, 0, 6:7])
                nc.vector.tensor_scalar(onePlus[:], onePlus[:], 1.0, None,
                                        op0=ALU.add)

            # ============ phase S: static gates + AllGather ============
            with tc.tile_pool(name="psS", bufs=1, space="PSUM") as psS:
                gs_ps = psS.tile([B, CSL], F32)
                # bias row opens the psum group (bias bcast to all 32 rows)
                nc.tensor.matmul(gs_ps[:], ones_bf[0:1, :], wb_sb[0:1, :],
                                 start=True, stop=False)
                for kc in range(KST):
                    nc.tensor.matmul(gs_ps[:], Xst_sb[:, kc, :], Wst_sb[:, kc, :],
                                     start=False, stop=(kc == KST - 1))
                nc.vector.tensor_copy(gs_loc[:], gs_ps[:])
            # by the time the in-order SP queue reaches this, gs_loc is ready
            nc.sync.dma_start(gin_b[:], gs_loc[:])
            nc.gpsimd.collective_compute(
                "AllGather", ALU.bypass,
                replica_groups=[list(range(N_CORES))],
                ins=[gin_b.opt()], outs=[gout_b.opt()])

            # all remaining proj before any feats: every batch's
            # logits/exp/alphaT work retires early and the tail is paced by
            # feats+W_fc arrival, not by the proj3 softmax chain
            for b in range(1, B_LOC):
                nc.sync.dma_start(projt[b][:],
                                  projT[b].rearrange("(c p) l -> p c l", p=P))
            for b in range(B_LOC):
                nc.sync.dma_start(fq[b][:],
                                  feats[b].rearrange("(j p) d -> p j d", p=P))
            # collective readback before the W_fc stream: the static-gates
            # inject opens the psum regions, so gs must land before the
            # first fc matmul could run
            nc.sync.dma_start(
                gs_sb[:].rearrange("b (i n) -> b i n", i=N_CORES),
                gout_b[:].rearrange("(i b) n -> b i n", i=N_CORES))
            for k in range(DC - 1):
                nc.sync.dma_start(WfcT_sb[k][:], WfcT[k * P:(k + 1) * P, :])
            for cc in (1, 3, 0, 2):    # tail needs f, then g, i; o last
                nc.sync.dma_start(Wfc3_sb[cc][:],
                                  WfcT[3 * P:4 * P, cc * 512:(cc + 1) * 512])

            # ============ phase B: attention, both halves pipelined ========
            # logits are produced TRANSPOSED: DR matmuls with the hatt chunk
            # as the stationary pair give lgT[128l, (h,lcp,m,b)] at psum
            # partition base 0 (DoubleRow dst must be partition 0). exp then
            # runs on [128, 8] tiles and writes the fp8 alphaT directly; the
            # mask is a tiny additive DVE op on the transposed logits.
            with tc.tile_pool(name="pslog", bufs=1, space="PSUM") as pslog, \
                 tc.tile_pool(name="pssum", bufs=1, space="PSUM") as pssum, \
                 tc.tile_pool(name="psctx", bufs=3, space="PSUM") as psctx, \
                 tc.tile_pool(name="psfc", bufs=1, space="PSUM") as psfc:
                lgT = pslog.tile([P, 2, LC // 2, 2, B_LOC], F32)
                sums_ps = pssum.tile([1, 2, B_LOC], F32)
                fcT_ps = psfc.tile([P, DC, B_LOC], F32)

                # zero-openers (complete groups; everything accumulates on
                # top with skip_group_check)
                nc.tensor.matmul(
                    lgT[:].rearrange("p a b c d -> p (a b c d)"),
                    ones128[0:1, :], zeros512[0:1, 0:2 * LC * B_LOC],
                    start=True, stop=True)
                nc.tensor.matmul(
                    sums_ps[:].rearrange("p a b -> p (a b)"),
                    ones_bf[0:1, 0:1], zeros512[0:1, 0:2 * B_LOC],
                    start=True, stop=True)
                nc.tensor.matmul(
                    fcT_ps[:].rearrange("p a b -> p (a b)"), ones128[0:1, :],
                    zeros512[0:1, 0:DC * B_LOC], start=True, stop=True)

                def relu_logits(h, b):
                    hatt8 = hattp.tile([P, DC, HALF], FP8)
                    for dc in range(DC):
                        src = projt[b][:, dc, h * HALF:(h + 1) * HALF]
                        qcol = qb[:, dc * B_LOC + b:dc * B_LOC + b + 1]
                        # per-batch balance: b0 avoids Pool (its queue is
                        # head-blocked by the collective's input wait); b3
                        # mostly avoids Pool so the last chain stays short
                        if b == 0:
                            eng = (("v", "a", "a", "v"), ("a", "v", "v", "a"))[h][dc]
                        elif b == B_LOC - 1:
                            eng = (("v", "a", "p", "v"), ("a", "v", "a", "p"))[h][dc]
                        elif h == 0:
                            eng = ("v", "p", "a", "v")[dc]
                        else:
                            eng = ("p", "v", "a", "p")[dc]
                        if eng == "p":
                            nc.gpsimd.tensor_scalar(
                                hatt8[:, dc, :], src, qcol, 0.0,
                                op0=ALU.add, op1=ALU.max)
                        elif eng == "a":
                            nc.scalar.activation(hatt8[:, dc, :], src,
                                                 AF.Relu, bias=qcol)
                        else:
                            nc.vector.tensor_scalar(
                                hatt8[:, dc, :], src, qcol, 0.0,
                                op0=ALU.add, op1=ALU.max)
                    # transposed logits: stationary = hatt d-pair x 128 l
                    # cols, moving = the padded w pair column
                    for lcp in range(LC // 2):
                        for m in range(2):
                            lc = 2 * lcp + m
                            for dcp in range(2):
                                nc.tensor.matmul(
                                    lgT[:, h, lcp, m, b:b + 1],
                                    hatt8[:, 2 * dcp:2 * dcp + 2,
                                          lc * P:(lc + 1) * P],
                                    wdr_sb[:, h, dcp, :, 0:1],
                                    start=False, stop=(dcp == 1),
                                    skip_group_check=True,
                                    perf_mode=DR)

                def softmax_ctx(b):
                    # mask, exp->fp8 alphaT, beta/sum fold, unnormalized ctx,
                    # scaled fc rows, accumulating transposes into fcT; all
                    # per-batch rows live at partition 0 now
                    for h in range(2):
                        nc.vector.tensor_tensor(
                            lgT[:, h, :, :, b], lgT[:, h, :, :, b],
                            maskT_sb[:, h, :, :, b], op=ALU.add)
                        nc.scalar.activation(
                            alphaT[:, 4 * h:4 * h + 4, :, b:b + 1],
                            lgT[:, h, :, :, b:b + 1],
                            AF.Exp, scale=1.0 / WSCALE)
                        for j in range(LC // 2):
                            nc.tensor.matmul(
                                sums_ps[0:1, h, b:b + 1],
                                onesdr[:, :, 0:1],
                                alphaT[:, 4 * h + j, :, b:b + 1],
                                start=False, stop=(j == LC // 2 - 1),
                                skip_group_check=True, perf_mode=DR)
                    nc.vector.tensor_scalar(sv[0:1, :, b], sums_ps[0:1, :, b],
                                            onePlus[0:1, b:b + 1], None,
                                            op0=ALU.mult)
                    nc.vector.reciprocal(sv[0:1, :, b], sv[0:1, :, b])
                    for h, row in ((0, fcrow), (1, fcrow_t)):
                        ctx_t = psctx.tile([1, D], F32, name="ctxt")
                        for j in range(LC // 2):
                            nc.tensor.matmul(
                                ctx_t[:],
                                alphaT[:, 4 * h + j, :, b:b + 1],
                                fq[b][:, 8 * h + 2 * j:8 * h + 2 * j + 2, :],
                                start=(j == 0), stop=(j == LC // 2 - 1),
                                perf_mode=DR)
                        if h == 0:
                            # ACT has slack now; DVE was pacing the chain
                            nc.scalar.activation(row[:], ctx_t[:], AF.Copy,
                                                 scale=sv[0:1, h, b:b + 1])
                        else:
                            nc.vector.tensor_scalar(row[:], ctx_t[:],
                                                    sv[0:1, h, b:b + 1], None,
                                                    op0=ALU.mult)
                        for dc in range(DC):
                            nc.tensor.matmul(
                                fcT_ps[:, dc, b:b + 1],
                                row[0:1, dc * P:(dc + 1) * P],
                                identf[:], is_transpose=True,
                                start=False, stop=(h == 1),
                                skip_group_check=True)

                for b in range(B_LOC):
                    if b >= 1:
                        softmax_ctx(b - 1)
                    relu_logits(0, b)
                    relu_logits(1, b)
                softmax_ctx(B_LOC - 1)
                nc.vector.tensor_copy(fcT_sb[:], fcT_ps[:])
                # all Exp-table ACT work is done; a dummy Sigmoid op
                # (reading the last exp's output so the scheduler pins it
                # right here) hoists the Sigmoid table load off the tail
                nc.scalar.activation(sigwarm[:], alphaT[0:1, 7, 1, 3:4],
                                     AF.Sigmoid)

            # streaming pools close here; the tail pool reuses their SBUF
            stream_ctx.close()

            # ============ phase C: gates + LSTM tail ============
            with tc.tile_pool(name="psg", bufs=1, space="PSUM") as psgp, \
                 tc.tile_pool(name="tailp", bufs=1) as tailp:
                # one psum tile per gate segment: each sigmoid then waits
                # only on its own segment's matmuls (deps are tile-granular)
                psgt = [psgp.tile([B_LOC, 512], F32, name=f"psg{c}")
                        for c in range(4)]
                for cc in (1, 3, 0, 2):   # close f, then g, i, o (tail order)
                    nc.tensor.matmul(psgt[cc][:], sel_sb[:],
                                     gs_sb[:, cc * 512:(cc + 1) * 512],
                                     start=True, stop=True)
                    for kc in range(DC):
                        mv = (WfcT_sb[kc][:, cc * 512:(cc + 1) * 512]
                              if kc < DC - 1 else Wfc3_sb[cc][:])
                        nc.tensor.matmul(
                            psgt[cc][:], fcT_sb[:, kc, :], mv,
                            start=False, stop=(kc == DC - 1),
                            skip_group_check=True)

                # gate columns are host-permuted to [i, f, o, g]
                g_sb = tailp.tile([B_LOC, G4], BF16)
                nc.scalar.activation(g_sb[:, H:2 * H], psgt[1][:],
                                     AF.Sigmoid)
                c1 = tailp.tile([B_LOC, H], F32)
                nc.vector.tensor_tensor(c1[:], g_sb[:, H:2 * H],
                                        c_last_sb[:], op=ALU.mult)
                # tanh(x) = 2*sigmoid(2x) - 1 (stays on the Sigmoid table)
                nc.scalar.activation(g_sb[:, 3 * H:4 * H], psgt[3][:],
                                     AF.Sigmoid, scale=2.0)
                nc.vector.tensor_scalar(g_sb[:, 3 * H:4 * H],
                                        g_sb[:, 3 * H:4 * H], 2.0, -1.0,
                                        op0=ALU.mult, op1=ALU.add)
                nc.scalar.activation(g_sb[:, 0:H], psgt[0][:], AF.Sigmoid)
                nc.scalar.activation(g_sb[:, 2 * H:3 * H], psgt[2][:],
                                     AF.Sigmoid)

                t2 = tailp.tile([B_LOC, H], BF16)
                tf = tailp.tile([B_LOC, H], F32)
                c_new = tailp.tile([B_LOC, H], F32)
                h_new = tailp.tile([B_LOC, H], F32)
                nc.vector.tensor_tensor(t2[:], g_sb[:, 0:H],
                                        g_sb[:, 3 * H:4 * H], op=ALU.mult)
                nc.vector.tensor_tensor(c_new[:], c1[:], t2[:], op=ALU.add)
                nc.sync.dma_start(c_out[:], c_new[:])
                nc.scalar.activation(tf[:], c_new[:], AF.Sigmoid, scale=2.0)
                nc.vector.tensor_scalar(tf[:], tf[:], 2.0, -1.0,
                                        op0=ALU.mult, op1=ALU.add)
                nc.vector.tensor_tensor(h_new[:], g_sb[:, 2 * H:3 * H], tf[:],
                                        op=ALU.mult)
                nc.scalar.dma_start(h_out[:], h_new[:])

    nc.compile()
    return nc


_NC_CACHE = None


def _get_nc():
    global _NC_CACHE
    if _NC_CACHE is None:
        _NC_CACHE = build_nc()
    return _NC_CACHE


def make_in_maps(features, features_proj, hidden_states, cell_states,
                 caption_hidden_states, w_h2a, b_h2a, w_patt, b_patt,
                 w_fatt, b_fatt, w_sel, b_sel, w_ih, w_hh, b_ih, b_hh,
                 mask, feature_idx):
    assert int(feature_idx) == FIDX
    import ml_dtypes
    f32 = np.float32
    bf16 = ml_dtypes.bfloat16
    fp8 = ml_dtypes.float8_e4m3
    features = np.asarray(features, f32)
    features_proj = np.asarray(features_proj, f32)
    h_last = np.asarray(hidden_states, f32)[-1]          # [B, H]
    c_lastv = np.asarray(cell_states, f32)[-1]           # [B, H]
    cap = np.asarray(caption_hidden_states, f32)         # [B, H]
    mask = np.asarray(mask)

    # fused LSTM weight, gate columns permuted [i, f, o, g]
    Wfull = np.concatenate([np.asarray(w_ih, f32), np.asarray(w_hh, f32)],
                           axis=1)                       # [2048c, 2048k]
    gate_perm = np.r_[0:512, 512:1024, 1536:2048, 1024:1536]
    WTp = np.ascontiguousarray(Wfull[gate_perm].T)       # [2048k, 2048c]
    biasv = (np.asarray(b_ih, f32) + np.asarray(b_hh, f32))[gate_perm]
    st_rows = np.r_[0:512, 1024:2048]                    # cap | feat | h rows
    WfcT = np.ascontiguousarray(WTp[512:1024]).astype(bf16)
    Wstat = WTp[st_rows]                                 # [1536, 2048]
    # static x for ALL batches: [capT; featT; hT]  [1536, 32]
    XstV = np.concatenate([cap.T, features[:, FIDX, :].T, h_last.T],
                          axis=0).astype(bf16)

    w_h2aTv = np.ascontiguousarray(
        np.asarray(w_h2a, f32).T * WSCALE).astype(fp8)
    # DR-padded attention weights: [p, h, dcp, m, 16], value at slot 0
    wdrv = np.zeros((P, 2, 2, 2, 16), f32)
    watt = np.stack([np.asarray(w_patt, f32)[0], np.asarray(w_fatt, f32)[0]])
    for h in range(2):
        for dcp in range(2):
            for m in range(2):
                dc = 2 * dcp + m
                wdrv[:, h, dcp, m, 0] = watt[h, dc * P:(dc + 1) * P] * WSCALE
    wdrv = wdrv.reshape(P, -1).astype(fp8)

    # additive mask in transposed-logit layout [p, h, lcp, m, b]; the
    # per-half attention bias b_att cancels in the softmax and is dropped
    madd = np.where(mask.reshape(B, 2, HALF), f32(0.0), f32(-1e30)) * WSCALE
    # [B, h, lcp, m, p] -> [p, h, lcp, m, B]
    maskTv = madd.reshape(B, 2, LC // 2, 2, P).transpose(4, 1, 2, 3, 0)

    in_maps = []
    for c in range(N_CORES):
        sl = slice(c * B_LOC, (c + 1) * B_LOC)
        packv = np.zeros((D, 7), f32)
        packv[:, 0:4] = h_last[sl].T
        packv[:, 4] = np.asarray(w_sel, f32)[0]
        packv[:, 5] = np.asarray(b_h2a, f32)
        packv[:, 6] = -np.asarray(b_sel, f32)[0]
        selv = np.zeros((B, B_LOC), f32)
        for j in range(B_LOC):
            selv[c * B_LOC + j, j] = 1.0
        in_maps.append({
            "projT": np.ascontiguousarray(
                features_proj[sl].transpose(0, 2, 1)).astype(fp8),
            "feats": np.ascontiguousarray(features[sl]).astype(fp8),
            "WfcT": WfcT,
            "Wst": np.ascontiguousarray(
                Wstat[:, c * CSL:(c + 1) * CSL]).astype(bf16),
            "Xst": XstV,
            "wbias": np.ascontiguousarray(
                biasv[None, c * CSL:(c + 1) * CSL]).astype(bf16),
            "sel": selv.astype(bf16),
            "w_h2aT": w_h2aTv,
            "wdr": wdrv,
            "pack": np.ascontiguousarray(packv).astype(bf16),
            "maskT": np.ascontiguousarray(
                maskTv[:, :, :, :, sl].reshape(P, -1)).astype(bf16),
            "c_last": np.ascontiguousarray(c_lastv[sl]),
        })
    return in_maps


def run(trace=False, **inputs):
    nc = _get_nc()
    in_maps = make_in_maps(**inputs)
    res = run_bass_kernel_spmd(nc, in_maps, core_ids=list(range(N_CORES)),
                               trace=trace)
    h = np.concatenate([res.results[c]["h_new"] for c in range(N_CORES)], axis=0)
    c = np.concatenate([res.results[c]["c_new"] for c in range(N_CORES)], axis=0)
    return (h[None], c[None]), res


def kernel(**inputs):
    out, _ = run(trace=False, **inputs)
    return out
